# revision 38
# baseline (speedup 1.0000x reference)
"""Multi-head attention Bass kernel for TRN2, 8 NeuronCores.

Sharding: batch (B=4) x head-half (2x8 heads) -> 8 cores.
Core c handles b = c//2, heads h0 = 8*(c%2) .. h0+8.

Per-core program (Tile framework):
  phase 0: load X[b], PE-transpose -> XT (d on partitions, float32r)
  phase 1: per head-pair: QT/KT projections (k on partitions, bias folded
           into DVE evict), V via VT + PE transpose back to [t, k]
  phase 2: per head, two streams:
    B (q on partitions): scores -> ACT exp(scale=1/8, accum_out row sums)
      -> 1/S -> normalize (DVE/GPSIMD tensor_scalar) -> DMA weights out
    A (t on partitions): transposed scores -> ACT exp (f32r) -> A.V
      accumulated in PSUM; normalized by broadcast 1/S at evict into concatT
  phase 3: out_partial = concatT.T @ Wo (K=64 steps), DMA out.
Host: scatters weights shards, sums the two partial outs per batch + bo.
"""

import sys
for _p in ("/opt/trn_rl_repo", "/root/.axon_site/_ro/trn_rl_repo"):
    if _p not in sys.path:
        sys.path.insert(0, _p)

import numpy as np
from contextlib import ExitStack

B, T, D, H, HD = 4, 2048, 1024, 16, 64
P = 128
NT = T // P        # 16 t-tiles
ND = D // P        # 8 d-tiles
NPAIR = 4          # head pairs per core
SCALE = 0.125      # 1/sqrt(HD)

_PROGRAM = None


def _build_program():
    import concourse.bacc as bacc
    import concourse.tile as tile
    from concourse import mybir

    f32 = mybir.dt.float32
    f32r = mybir.dt.float32r
    AF = mybir.ActivationFunctionType

    nc = bacc.Bacc("TRN2", target_bir_lowering=False, debug=False)

    Xb_d = nc.dram_tensor("Xb", [T, D], f32, kind="ExternalInput")
    Wq_d = nc.dram_tensor("Wq", [NPAIR, P, D], f32r, kind="ExternalInput")
    Wk_d = nc.dram_tensor("Wk", [NPAIR, P, D], f32r, kind="ExternalInput")
    Wv_d = nc.dram_tensor("Wv", [NPAIR, P, D], f32r, kind="ExternalInput")
    bqk_d = nc.dram_tensor("bqk", [P, 2 * NPAIR], f32, kind="ExternalInput")
    bv_d = nc.dram_tensor("bv", [P, NPAIR], f32, kind="ExternalInput")
    Wo_d = nc.dram_tensor("Wo", [4, P, D], f32r, kind="ExternalInput")
    ident_d = nc.dram_tensor("ident", [P, P], f32, kind="ExternalInput")
    identr_d = nc.dram_tensor("identr", [P, P], f32r, kind="ExternalInput")
    Wout_d = nc.dram_tensor("Wout", [8, T, T], f32, kind="ExternalOutput")
    Oout_d = nc.dram_tensor("Oout", [T, D], f32, kind="ExternalOutput")

    with tile.TileContext(nc) as tc, ExitStack() as top:
        const_pool = top.enter_context(tc.tile_pool(name="const", bufs=1))
        ident = const_pool.tile([P, P], f32)
        nc.sync.dma_start(ident[:], ident_d.ap())
        identr = const_pool.tile([P, P], f32r)
        nc.sync.dma_start(identr[:], identr_d.ap())
        bqk_sb = const_pool.tile([P, 2 * NPAIR], f32)
        nc.sync.dma_start(bqk_sb[:], bqk_d.ap())
        bv_sb = const_pool.tile([P, NPAIR], f32)
        nc.sync.dma_start(bv_sb[:], bv_d.ap())

        # persistent SBUF for the whole head phase
        qk_pool = top.enter_context(tc.tile_pool(name="qk", bufs=2 * NPAIR))
        vp_pool = top.enter_context(tc.tile_pool(name="vp", bufs=8))

        QT = [qk_pool.tile([P, T], f32r, tag="qk", name=f"QT{i}") for i in range(NPAIR)]
        KT = [qk_pool.tile([P, T], f32r, tag="qk", name=f"KT{i}") for i in range(NPAIR)]
        # V per head: [t-local 128, 16 tiles x 64 cols] (f32r)
        Vh = [vp_pool.tile([P, NT * HD], f32r, tag="vp", name=f"Vh{i}") for i in range(8)]

        # ---------------- phase 0 + 1: XT build and projections ----------
        with tc.tile_pool(name="xt", bufs=ND) as xt_pool:
            XT = [xt_pool.tile([P, T], f32r, tag="xt", name=f"XT{i}") for i in range(ND)]
            with tc.tile_pool(name="xn", bufs=6) as xn_pool, \
                 tc.tile_pool(name="tp_ps", bufs=2, space="PSUM") as tp_psum:
                for tb in range(NT // 4):
                    xns = []
                    for j in range(4):
                        tt = tb * 4 + j
                        xn = xn_pool.tile([P, D], f32, tag="xn")
                        nc.sync.dma_start(
                            xn[:], Xb_d.ap()[tt * P:(tt + 1) * P, :])
                        xns.append(xn)
                    for dt in range(ND):
                        ps = tp_psum.tile([P, 512], f32, tag="tp")
                        for j in range(4):
                            nc.tensor.transpose(
                                ps[:, j * P:(j + 1) * P],
                                xns[j][:, dt * P:(dt + 1) * P],
                                ident[:])
                        nc.vector.tensor_copy(
                            XT[dt][:, tb * 512:(tb + 1) * 512], ps[:])

            # projections per pair
            with tc.tile_pool(name="wts", bufs=3) as w_pool, \
                 tc.tile_pool(name="vtsb", bufs=2) as vt_pool, \
                 tc.tile_pool(name="pj_ps", bufs=4, space="PSUM") as pj_psum, \
                 tc.tile_pool(name="vt_ps", bufs=2, space="PSUM") as vt_psum:
                for pr in range(NPAIR):
                    wq = w_pool.tile([P, D], f32r, tag="w")
                    wk = w_pool.tile([P, D], f32r, tag="w")
                    wv = w_pool.tile([P, D], f32r, tag="w")
                    nc.sync.dma_start(wq[:], Wq_d.ap()[pr])
                    nc.sync.dma_start(wk[:], Wk_d.ap()[pr])
                    nc.sync.dma_start(wv[:], Wv_d.ap()[pr])
                    vt_sb = vt_pool.tile([P, T], f32r, tag="vt")
                    for tch in range(4):
                        sl = slice(tch * 512, (tch + 1) * 512)
                        psq = pj_psum.tile([P, 512], f32, tag="pj")
                        psk = pj_psum.tile([P, 512], f32, tag="pj")
                        psv = pj_psum.tile([P, 512], f32, tag="pj")
                        for dt in range(ND):
                            dsl = slice(dt * P, (dt + 1) * P)
                            st = dict(start=(dt == 0), stop=(dt == ND - 1))
                            nc.tensor.matmul(psq[:], wq[:, dsl], XT[dt][:, sl], **st)
                            nc.tensor.matmul(psk[:], wk[:, dsl], XT[dt][:, sl], **st)
                            nc.tensor.matmul(psv[:], wv[:, dsl], XT[dt][:, sl], **st)
                        nc.vector.tensor_scalar_add(
                            QT[pr][:, sl], psq[:], bqk_sb[:, 2 * pr:2 * pr + 1])
                        nc.vector.tensor_scalar_add(
                            KT[pr][:, sl], psk[:], bqk_sb[:, 2 * pr + 1:2 * pr + 2])
                        nc.vector.tensor_scalar_add(
                            vt_sb[:, sl], psv[:], bv_sb[:, pr:pr + 1])
                    # transpose VT -> per-head V [t, k]
                    for tb in range(NT // 4):
                        vps = vt_psum.tile([P, 512], f32r, tag="vtp")
                        for j in range(4):
                            tt = tb * 4 + j
                            nc.tensor.transpose(
                                vps[:, j * P:(j + 1) * P],
                                vt_sb[:, tt * P:(tt + 1) * P],
                                identr[:])
                        for hh in range(2):
                            # strided copy: 4 blocks of 64 cols -> Vh
                            dst = Vh[2 * pr + hh][:].rearrange(
                                "p (n k) -> p n k", k=HD)[:, tb * 4:tb * 4 + 4, :]
                            src = vps[:].rearrange(
                                "p (n k) -> p n k", k=HD)[:, hh::2, :]
                            nc.vector.tensor_copy(dst, src)

        # ---------------- phase 2: per-head streams ----------------------
        cat_pool = top.enter_context(tc.tile_pool(name="cat", bufs=4))
        concatT = [cat_pool.tile([P, T], f32r, tag="cat", name=f"catT{i}")
                   for i in range(4)]
        with tc.tile_pool(name="eb", bufs=3) as eb_pool, \
             tc.tile_pool(name="et", bufs=2) as et_pool, \
             tc.tile_pool(name="wtile", bufs=2) as wt_pool, \
             tc.tile_pool(name="sacc", bufs=4) as sacc_pool, \
             tc.tile_pool(name="rh", bufs=2) as rh_pool, \
             tc.tile_pool(name="rb", bufs=1) as rb_pool, \
             tc.tile_pool(name="avt", bufs=1) as avt_pool, \
             tc.tile_pool(name="r1", bufs=2) as r1_pool, \
             tc.tile_pool(name="pb_ps", bufs=1, space="PSUM") as pb_psum, \
             tc.tile_pool(name="pa_ps", bufs=1, space="PSUM") as pa_psum, \
             tc.tile_pool(name="av_ps", bufs=2, space="PSUM") as av_psum:
            for h in range(8):
                pr, hh = h // 2, h % 2
                pp = slice(hh * HD, hh * HD + HD)
                R_h = rh_pool.tile([P, NT], f32, tag="rh")
                avtmp = (avt_pool.tile([HD, T], f32r, tag="avt", name=f"avt{h}")
                         if hh else None)
                # ---- B stream: weights output -------------------------
                for qt2 in range(NT // 2):
                    w_tile = wt_pool.tile([P, 2 * T], f32, tag="w")
                    for qi in range(2):
                        qt = qt2 * 2 + qi
                        qsl = slice(qt * P, (qt + 1) * P)
                        sacc = sacc_pool.tile([P, 3], f32, tag="sacc")
                        ebs = []
                        for half in range(2):
                            pb = pb_psum.tile([P, 1024], f32, tag="pb")
                            for c2 in range(2):
                                tsl = slice((half * 2 + c2) * 512,
                                            (half * 2 + c2) * 512 + 512)
                                nc.tensor.matmul(
                                    pb[:, c2 * 512:(c2 + 1) * 512],
                                    QT[pr][pp, qsl], KT[pr][pp, tsl],
                                    start=True, stop=True)
                            eb = eb_pool.tile([P, 1024], f32, tag="eb")
                            nc.scalar.activation(
                                eb[:], pb[:], AF.Exp, scale=SCALE,
                                accum_out=sacc[:, half:half + 1])
                            ebs.append(eb)
                        nc.vector.tensor_add(
                            sacc[:, 2:3], sacc[:, 0:1], sacc[:, 1:2])
                        nc.vector.reciprocal(R_h[:, qt:qt + 1], sacc[:, 2:3])
                        for half in range(2):
                            eng = nc.gpsimd if (2 * qi + half) % 2 == 0 else nc.vector
                            eng.tensor_scalar_mul(
                                w_tile[:, qi * T + half * 1024:
                                       qi * T + (half + 1) * 1024],
                                ebs[half][:], R_h[:, qt:qt + 1])
                    nc.sync.dma_start(
                        Wout_d.ap()[h].rearrange("(a p) t -> p a t", p=P)
                        [:, qt2 * 2:qt2 * 2 + 2, :],
                        w_tile[:].rearrange("p (a t) -> p a t", a=2))
                # R_h -> q-major row -> broadcast Rb [128, T]
                rt_ps = av_psum.tile([P, 512], f32, tag="av")
                nc.tensor.transpose(rt_ps[0:NT, 0:P], R_h[:], ident[:])
                r1t = r1_pool.tile([NT, P], f32, tag="r1t")
                nc.vector.tensor_copy(r1t[:], rt_ps[0:NT, 0:P])
                rb = rb_pool.tile([P, T], f32, tag="rb")
                nc.sync.dma_start(
                    rb[0:1, :].rearrange("p (a b) -> p a b", a=NT), r1t[:])
                nc.gpsimd.partition_broadcast(rb[:], rb[0:1, :])
                # ---- A stream: transposed scores -> exp -> A.V ----------
                for qc in range(4):
                    qsl = slice(qc * 512, (qc + 1) * 512)
                    pa = pa_psum.tile([P, T], f32, tag="pa")
                    av = av_psum.tile([P, 512], f32, tag="av")
                    for tb in range(4):
                        et = et_pool.tile([P, T], f32r, tag="et")
                        for jh in range(2):
                            for j in (2 * jh, 2 * jh + 1):
                                tt = tb * 4 + j
                                nc.tensor.matmul(
                                    pa[:, j * 512:(j + 1) * 512],
                                    KT[pr][pp, tt * P:(tt + 1) * P],
                                    QT[pr][pp, qsl], start=True, stop=True)
                            hsl = slice(jh * 1024, (jh + 1) * 1024)
                            nc.scalar.activation(
                                et[:, hsl], pa[:, hsl], AF.Exp, scale=SCALE)
                            for j in (2 * jh, 2 * jh + 1):
                                tt = tb * 4 + j
                                nc.tensor.matmul(
                                    av[0:HD, :],
                                    Vh[h][:, tt * HD:(tt + 1) * HD],
                                    et[:, j * 512:(j + 1) * 512],
                                    start=(tb == 0 and j == 0),
                                    stop=(tb == 3 and j == 3))
                    if hh == 0:
                        nc.vector.tensor_mul(
                            concatT[pr][0:HD, qsl], av[0:HD, :], rb[0:HD, qsl])
                    else:
                        nc.vector.tensor_mul(
                            avtmp[:, qsl], av[0:HD, :], rb[0:HD, qsl])
                if hh == 1:
                    nc.sync.dma_start(concatT[pr][HD:P, :], avtmp[:])

            # ---- output projection (reuses pb psum + wt/vp sbuf slots) --
            WoS = []
            for ct in range(4):
                wo = vp_pool.tile([P, D], f32r, tag="vp", name=f"WoS{ct}")
                nc.sync.dma_start(wo[:], Wo_d.ap()[ct])
                WoS.append(wo)
            for tt in range(NT):
                tsl = slice(tt * P, (tt + 1) * P)
                ps = pb_psum.tile([P, 1024], f32, tag="pb")
                osb = wt_pool.tile([P, D], f32, tag="w", name=f"osb{tt}")
                for nc2 in range(2):
                    nsl = slice(nc2 * 512, (nc2 + 1) * 512)
                    for ct in range(4):
                        nc.tensor.matmul(
                            ps[:, nsl], concatT[ct][:, tsl], WoS[ct][:, nsl],
                            start=(ct == 0), stop=(ct == 3))
                nc.vector.tensor_copy(osb[:], ps[:])
                nc.sync.dma_start(Oout_d.ap()[tsl, :], osb[:])

    nc.compile()
    return nc


def _get_program():
    global _PROGRAM
    if _PROGRAM is None:
        _PROGRAM = _build_program()
    return _PROGRAM


def kernel(X, Wq, bq, Wk, bk, Wv, bv, Wo, bo, _trace=False):
    from concourse.bass_utils import run_bass_kernel_spmd

    X = np.asarray(X, np.float32)
    Wq = np.asarray(Wq, np.float32)
    bq = np.asarray(bq, np.float32)
    Wk = np.asarray(Wk, np.float32)
    bk = np.asarray(bk, np.float32)
    Wv = np.asarray(Wv, np.float32)
    bv = np.asarray(bv, np.float32)
    Wo = np.asarray(Wo, np.float32)
    bo = np.asarray(bo, np.float32)

    ident = np.eye(P, dtype=np.float32)
    in_maps = []
    for c in range(8):
        b, h0 = c // 2, 8 * (c % 2)
        hs = slice(h0, h0 + 8)
        # [8, D, HD] -> pairs [4, D, 128] -> SBUF layout [4, P, ND*P]
        def _wprep(W):
            wp = W[hs].reshape(4, 2, D, HD).transpose(0, 2, 1, 3).reshape(4, D, P)
            return np.ascontiguousarray(
                wp.reshape(4, ND, P, P).transpose(0, 2, 1, 3).reshape(4, P, D))
        wq_c, wk_c, wv_c = _wprep(Wq), _wprep(Wk), _wprep(Wv)
        # [P, 2*NPAIR]: col 2*pr+j, j=0 bq / j=1 bk
        bqk_c = np.ascontiguousarray(
            np.stack([bq[hs].reshape(4, P), bk[hs].reshape(4, P)],
                     axis=-1).transpose(1, 0, 2).reshape(P, 2 * NPAIR))
        bv_c = np.ascontiguousarray(bv[hs].reshape(4, P).T)
        wo_c = np.ascontiguousarray(
            Wo[h0 * HD:(h0 + 8) * HD, :].reshape(4, P, D))
        in_maps.append({
            "Xb": np.ascontiguousarray(X[b]),
            "Wq": wq_c, "Wk": wk_c, "Wv": wv_c,
            "bqk": bqk_c, "bv": bv_c, "Wo": wo_c,
            "ident": ident, "identr": ident,
        })

    nc = _get_program()
    res = run_bass_kernel_spmd(nc, in_maps, list(range(8)), trace=_trace)

    weights = np.empty((B, H, T, T), np.float32)
    out = np.empty((B, T, D), np.float32)
    for c in range(8):
        b, h0 = c // 2, 8 * (c % 2)
        weights[b, h0:h0 + 8] = res.results[c]["Wout"]
    for b in range(B):
        out[b] = (res.results[2 * b]["Oout"] + res.results[2 * b + 1]["Oout"]
                  + bo[None, :])
    if _trace:
        kernel._last_exec_time_ns = res.exec_time_ns
        kernel._last_results = res
    return out, weights
